# revision 109
# baseline (speedup 1.0000x reference)
"""DiffusedFarthestAttention Trainium2 kernel (8-core SPMD Bass/Tile).

Decomposition (B=4 batches x 2 halves -> 8 cores; pair (2b, 2b+1) handles batch b):
  Phase 1: to_basis, N-split.  xspec_partial[K,C] = sum_n (evecs[n,:]*mass[n])^T x[n,:]
           over this core's 16384 rows (mass scale via TensorScalarPtr, DVE 4x
           mode on bf16).  All param/const loads ride the ACT HWDGE queue so
           the SP queue streams x/ev from t=0; the evfar Gram matrix + column
           sums (spectral GroupNorm stats) hide under P1's DMA.  coef_in is
           folded during PSUM evacuation so AllReduce #1 carries the finished
           pre-spectrum.
  Middle (head-split, 4 heads per core; all 8 programs identical, split lives
           in the DATA): GroupNorm stats spectrally from spec1 and the Gram
           matrix (rstd via DVE-only fast-inverse-sqrt, keeping ACT and its
           table loads off the chain; Exp is the kernel's single act table).
           The GN affine is folded into the QKV weights (per-channel
           TensorScalarPtr row scales) and biases (transposed W^T B matmuls),
           so x_farT stays raw and the stats chain runs concurrently with the
           x_farT matmuls.  Per-head scoresT land in single-bank [128,512]
           PSUM tiles (4-deep) -> back-to-back 512-wide exps keep ACT ~100%
           busy -> ones-augmented PV gives softmax denominators, broadcast
           across partitions by a ones-stationary matmul (no DRAM bounce).
           v bias + bo land as one precomputed bo2 row (attention rows sum to
           one), pre-broadcast; heads 0-2's out-projection partials are
           pre-accumulated in head 3's PE slack so the tail is one matmul and
           one add per chunk.
           zspec partial is pre-scaled by coef_out*out_w; AllReduce #2 carries
           the finished bf16 spectrum, directly consumable by phase 3.
  Phase 3: from_basis, N-split.  out rows = evT_chunk^T @ spec2, written bf16
           (halves the 16MB output stream); paired one-bank PSUM tiles with
           DVE/ACT alternating evacuation.  evT fully prefetched during the
           middle; per-partition-contiguous row-block layouts keep every DMA
           at >=2KB descriptors.

Heavy matmuls run as bfloat16 (P1/P3 streams) or float32r (FP22) elsewhere.
Host-side prep is layout-only (transposes, gathers by far_idx, reshapes, dtype
casts); all arithmetic happens on device.

PE warm-up matmuls run through both AllReduce windows so the post-AR chains
start at the fast p-state; evTfar comes from PE transposes of the evfar chunks
instead of its own 512KB DMA.

f32r end-to-end AllReduce #1 (DVE pre-rounds, so no post-AR re-rounding copy)
and a single-Newton fast-rsqrt keep the stats chain short.

Deep pool buffering (P1 stream 5, P3 psum 8 banks / staging 4) keeps the
DMA-bound phases streaming without pipeline stalls.

TimelineSim (single-core proxy): 145.7us; baseline was 204us.  HW reps-slope
matches sim within measurement noise.  rel err 6.5e-3 (budget 2e-2).
"""

import numpy as np

import concourse.bass as bass
import concourse.mybir as mybir
import concourse.tile as tile
from concourse import bacc
from concourse.bass_utils import run_bass_kernel_spmd

B, N, K, M = 4, 32768, 128, 1024
C = 256          # C_IN = C_OUT = C_ATT
H, D = 8, 32     # heads, head dim
EPS = 1e-6
P = 128
NH = N // 2      # rows per core
NCH = NH // P    # 128 n-chunks per core
P1G = 8          # n-chunks per P1 group
P3G = 8          # n-chunks per P3 group
HL = H // 2      # heads per core
NMK = M // P
F32 = mybir.dt.float32
F32R = mybir.dt.float32r
BF16 = mybir.dt.bfloat16
DT1 = BF16       # phase-1 stream dtype (x, evecs natural)
DT3 = BF16       # phase-3 stream dtype (evT, spec2w)
P3E_BUFS = 16 if DT3 == BF16 else 10
ADD = mybir.AluOpType.add
MULT = mybir.AluOpType.mult
AF = mybir.ActivationFunctionType

# packed f32 param columns (pk1)
_PK1 = dict(massT=(0, NCH), mfarT=(128, NMK), maskq=(136, HL), bk=(140, 1),
            rsqm=(142, 1), gnw=(144, 2), gnb=(146, 2), evals=(148, 1),
            bq=(149, 1), bvh=(150, HL))
PK1_W = 154
# packed f32r matrix columns (pkr)
_PKR = dict(gsum=(0, 16), gbp=(16, P))
PKR_W = 144
# packed f32r weight columns (pkw): wq0|wq1|wk0|wk1|wv0|wv1
PKW_W = 2 * P + 2 * P + 2 * C


def _build(single=False, phases=(1, 2, 3), reps=1, noar=False):
    """single=True: 1-core variant with AllReduce -> local copy, for TimelineSim."""
    nc = bacc.Bacc("TRN2", target_bir_lowering=False, debug=False,
                   enable_asserts=False, num_devices=1 if single else 8)
    dt = F32
    x_h = nc.dram_tensor("x_h", [NH, C], DT1, kind="ExternalInput").ap()
    ev_h = nc.dram_tensor("ev_h", [NH, K], DT1, kind="ExternalInput").ap()
    evT_h = nc.dram_tensor("evT_h", [K, NH], DT3, kind="ExternalInput").ap()
    evfar = nc.dram_tensor("evfar", [M, K], F32R, kind="ExternalInput").ap()
    ident = nc.dram_tensor("ident", [P, P], F32R, kind="ExternalInput").ap()
    pk1 = nc.dram_tensor("pk1", [P, PK1_W], F32, kind="ExternalInput").ap()
    pkr = nc.dram_tensor("pkr", [P, PKR_W], F32R, kind="ExternalInput").ap()
    pkw = nc.dram_tensor("pkw", [P, PKW_W], F32R, kind="ExternalInput").ap()
    pkwo = nc.dram_tensor("pkwo", [D, HL * C], F32R, kind="ExternalInput").ap()
    rows = nc.dram_tensor("rows", [6, C], F32, kind="ExternalInput").ap()
    konst = nc.dram_tensor("konst", [2, 512], F32R, kind="ExternalInput").ap()
    out_ap = nc.dram_tensor("out", [NH, C], BF16, kind="ExternalOutput").ap()

    RG = [[0, 1], [2, 3], [4, 5], [6, 7]]

    with tile.TileContext(nc) as tc:
        with tc.tile_pool(name="const", bufs=1) as cst, \
             tc.tile_pool(name="mid", bufs=4) as mid, \
             tc.tile_pool(name="p3e", bufs=P3E_BUFS) as p3e, \
             tc.tile_pool(name="dram", bufs=1, space="DRAM") as dram:
            for rep in range(reps):
                # ---- packed params: on the ACT HWDGE queue so the SP queue
                # starts the P1 x/ev stream immediately ----
                pk1_t = cst.tile([P, PK1_W], dt, tag="pk1")
                nc.scalar.dma_start(pk1_t[:], pk1[:])
                pkr_t = cst.tile([P, PKR_W], F32R, tag="pkr")
                nc.scalar.dma_start(pkr_t[:], pkr[:])
                pkw_t = cst.tile([P, PKW_W], F32R, tag="pkw")
                nc.scalar.dma_start(pkw_t[:], pkw[:])
                pkwo_t = cst.tile([D, HL * C], F32R, tag="pkwo")
                nc.scalar.dma_start(pkwo_t[:], pkwo[:])

                def p1(name):
                    o, w = _PK1[name]
                    return pk1_t[:, o:o + w]
                massT_t, mfarT_t = p1("massT"), p1("mfarT")
                maskq_t, bk_t, bvh_t = p1("maskq"), p1("bk"), p1("bvh")
                gnw_t, gnb_t = p1("gnw"), p1("gnb")
                evals_t, bq_t = p1("evals"), p1("bq")
                rsqm_t = pk1_t[0:16, _PK1["rsqm"][0]:_PK1["rsqm"][0] + 1]
                gsum_t = pkr_t[:, 0:16]
                gbp_t = pkr_t[:, 16:16 + P]
                wq_t = [pkw_t[:, j * P:(j + 1) * P] for j in range(2)]
                wk_t = [pkw_t[:, 2 * P + j * P:2 * P + (j + 1) * P] for j in range(2)]
                wv_t = [pkw_t[:, 4 * P + j * C:4 * P + (j + 1) * C] for j in range(2)]
                wo_t = [pkwo_t[0:D, h * C:(h + 1) * C] for h in range(HL)]

                ones2 = cst.tile([P, 2], F32R, tag="ones2")
                nc.scalar.dma_start(ones2[:], konst[0:1, 0:2].to_broadcast([P, 2]))
                zeros8 = cst.tile([P, 8], F32R, tag="zeros8")
                nc.scalar.dma_start(zeros8[:], konst[1:2, 256:264].to_broadcast([P, 8]))
                ones128 = cst.tile([1, P], F32R, tag="ones128")
                nc.scalar.dma_start(ones128[:], konst[0:1, 0:P])
                # row params broadcast over partitions (DMA does the broadcast)
                tin_b = cst.tile([P, C], dt, tag="tinb")
                nc.scalar.dma_start(tin_b[:], rows[0:1, :].to_broadcast([P, C]))
                nc.vector.tensor_scalar_max(tin_b[:], tin_b[:], 1e-8)
                tout_b = cst.tile([P, C], dt, tag="toutb")
                nc.scalar.dma_start(tout_b[:], rows[1:2, :].to_broadcast([P, C]))
                nc.vector.tensor_scalar_max(tout_b[:], tout_b[:], 1e-8)
                outw_b = cst.tile([P, C], dt, tag="outwb")
                nc.scalar.dma_start(outw_b[:], rows[2:3, :].to_broadcast([P, C]))
                nc.vector.tensor_scalar_max(outw_b[:], outw_b[:], 1e-8)
                bo_r = cst.tile([1, C], F32R, tag="bor")
                nc.scalar.dma_start(bo_r[:], konst[1:2, 0:C])

                # coefs = exp(-evals x t); coefow folds out_w into coef_out
                coef_in = cst.tile([P, C], dt, tag="coefin")
                nc.vector.tensor_tensor(coef_in[:], evals_t.to_broadcast([P, C]),
                                        tin_b[:], MULT)
                nc.scalar.activation(coef_in[:], coef_in[:], AF.Exp, scale=-1.0)
                coef_out = cst.tile([P, C], dt, tag="coefout")
                nc.vector.tensor_tensor(coef_out[:], evals_t.to_broadcast([P, C]),
                                        tout_b[:], MULT)
                nc.scalar.activation(coef_out[:], coef_out[:], AF.Exp, scale=-1.0)
                coefow = cst.tile([P, C], dt, tag="coefow")
                nc.vector.tensor_tensor(coefow[:], coef_out[:], outw_b[:], MULT)

                if 2 in phases:
                    # vaug [P, mc, h, D+1]: only the ones-columns need a fill;
                    # v blocks are written later from PSUM
                    vaug = cst.tile([P, NMK, HL, D + 1], F32R, tag="vaug")
                    for mc in range(NMK):
                        for h in range(HL):
                            nc.vector.tensor_copy(out=vaug[:, mc, h, D:D + 1],
                                                  in_=ones2[:, 0:1])

                # =============== PHASE 1: to_basis (N-split) ===============
                with tc.tile_pool(name="p1x", bufs=5) as p1x, \
                     tc.tile_pool(name="p1e", bufs=5) as p1e, \
                     tc.tile_pool(name="ps1", bufs=1, space="PSUM") as ps1:
                    if 2 in phases:
                        # evfar: one load; Gram + column sums for spectral GN;
                        # then mass-scale in place (zspec use)
                        gram_ps = ps1.tile([K, K], dt, tag="gram")
                        s_ps = ps1.tile([K, 2], dt, tag="sps")
                        ef_all = cst.tile([P, NMK, K], F32R, tag="efall")
                        nc.scalar.dma_start(
                            ef_all[:], evfar[:, :].rearrange("(m p) k -> p m k", p=P))
                        for mc in range(NMK):
                            nc.tensor.matmul(gram_ps[:], ef_all[:, mc, :], ef_all[:, mc, :],
                                             start=(mc == 0), stop=(mc == NMK - 1))
                            nc.tensor.matmul(s_ps[:], ef_all[:, mc, :], ones2[:, 0:2],
                                             start=(mc == 0), stop=(mc == NMK - 1))
                        # evTfar via PE transpose of the (still unscaled)
                        # evfar chunks: saves the 512KB evTfar DMA; hides
                        # under P1's stream
                        ident_t = cst.tile([P, P], F32R, tag="ident")
                        nc.scalar.dma_start(ident_t[:], ident[:])
                        evTfar_t = cst.tile([K, M], F32R, tag="evTfar")
                        for mc in range(NMK):
                            tp = ps1.tile([P, P], F32R, tag="tp", bufs=2)
                            nc.tensor.transpose(tp[:], ef_all[:, mc, :], ident_t[:])
                            if mc % 2 == 0:
                                nc.vector.tensor_copy(
                                    out=evTfar_t[:, mc * P:(mc + 1) * P], in_=tp[:])
                            else:
                                nc.scalar.copy(evTfar_t[:, mc * P:(mc + 1) * P],
                                               tp[:])
                        for mc in range(NMK):
                            nc.vector.tensor_scalar_mul(ef_all[:, mc, :], ef_all[:, mc, :],
                                                        mfarT_t[:, mc:mc + 1])
                        gram_sb = cst.tile([K, K], F32R, tag="gram_sb")
                        nc.scalar.copy(gram_sb[:], gram_ps[:])
                        s_sb = cst.tile([K, 2], F32R, tag="s_sb")
                        nc.vector.tensor_copy(out=s_sb[:], in_=s_ps[:])

                    xspec_ps = ps1.tile([K, C], dt, tag="xspec")
                    ng = NCH // P1G
                    for g in range(ng):
                        # per-partition contiguous row blocks: [p, j] = row p*P1G+j
                        # (evecs load issues first: the mass-scale chains off it)
                        et = p1e.tile([P, P1G, K], DT1, tag="e8")
                        nc.sync.dma_start(
                            et[:], ev_h[g * P1G * P:(g + 1) * P1G * P, :]
                            .rearrange("(p j) k -> p j k", j=P1G))
                        xt = p1x.tile([P, P1G, C], DT1, tag="x8")
                        nc.sync.dma_start(
                            xt[:], x_h[g * P1G * P:(g + 1) * P1G * P, :]
                            .rearrange("(p j) c -> p j c", j=P1G))
                        for j in range(P1G):
                            # 2D per-chunk scale: matmul j waits ~100ns, not the
                            # whole group's 3D scale.  TensorScalarPtr hits the
                            # DVE 4x mode on packed bf16.
                            cix = g * P1G + j
                            nc.vector.tensor_scalar_mul(
                                et[:, j, :], et[:, j, :], massT_t[:, cix:cix + 1])
                            nc.tensor.matmul(xspec_ps[:], et[:, j, :], xt[:, j, :],
                                             start=(g == 0 and j == 0),
                                             stop=(g == ng - 1 and j == P1G - 1))
                    # fold coef_in during the PSUM evacuation: AR carries the
                    # pre-scaled spectrum, so nothing runs post-AR before use
                    spec1loc = cst.tile([K, C], F32R, tag="spec1loc")
                    nc.vector.tensor_tensor(spec1loc[:], coef_in[:], xspec_ps[:],
                                            MULT)

                # AllReduce #1 (pair); evT prefetch issues right after so the
                # transfers fill the middle phase's otherwise-idle DMA
                ar1_in = dram.tile([K, C], F32R, tag="ar1in")
                ar1_out = dram.tile([K, C], F32R, tag="ar1out")
                nc.sync.dma_start(ar1_in[:], spec1loc[:])
                if single or noar:
                    nc.sync.dma_start(ar1_out[:], ar1_in[:])
                else:
                    nc.gpsimd.collective_compute(
                        "AllReduce", ADD, replica_groups=RG,
                        ins=[ar1_in[:].opt()], outs=[ar1_out[:].opt()])
                # f32r AR end-to-end: the DVE-written (rounded) spectrum
                # round-trips the collective bit-identically, so the middle
                # consumes the DMA'd tile with no re-rounding copy
                spec1 = cst.tile([K, C], F32R, tag="spec1")
                nc.sync.dma_start(spec1[:], ar1_out[:])
                if 3 in phases:
                    p3et = [p3e.tile([K, P3G * P], DT3, tag="evt8", bufs=P3E_BUFS,
                                     name=f"p3et{g}") for g in range(NCH // P3G)]
                    for g in range(P3E_BUFS):
                        nc.sync.dma_start(p3et[g][:],
                                          evT_h[:, g * P3G * P:(g + 1) * P3G * P])

                if 2 in phases:
                    # =============== MIDDLE ===============
                    with tc.tile_pool(name="psm", bufs=1, space="PSUM") as psm:
                        # PE warm-up through the AR1 window: dependency-free
                        # matmuls keep the tensor engine continuously busy so
                        # the post-AR chain starts at the fast p-state instead
                        # of cold (1.54 -> 0.42 ns/col)
                        for w in range(56):
                            wt = psm.tile([P, C], dt, tag="mm256", bufs=2,
                                          name=f"warm1_{w}")
                            nc.tensor.matmul(wt[:], ef_all[:, w % NMK, :],
                                             evTfar_t[:, 0:C],
                                             start=True, stop=True)
                        # ---- spectral GN stats; GN affine is folded into the
                        # QKV weights/biases so x_farT stays raw ----
                        t1 = psm.tile([K, C], dt, tag="mm256", bufs=2)
                        nc.tensor.matmul(t1[:], gram_sb[:], spec1[:],
                                         start=True, stop=True)
                        sq = cst.tile([K, C], F32R, tag="sq")
                        nc.vector.tensor_tensor(sq[:], spec1[:], t1[:], MULT)
                        # fp32r matmuls need even free dims: stats come out as
                        # duplicated column pairs, compacted below
                        stat_ps = psm.tile([P, 8], dt, tag="mm256", bufs=2)
                        for cc in range(2):
                            nc.tensor.matmul(stat_ps[:, 2 * cc:2 * cc + 2],
                                             spec1[:, cc * P:(cc + 1) * P], s_sb[:],
                                             start=True, stop=True)
                            nc.tensor.matmul(stat_ps[:, 4 + 2 * cc:6 + 2 * cc],
                                             sq[:, cc * P:(cc + 1) * P], ones2[:, 0:2],
                                             start=True, stop=True)

                        # ---- x_farT raw (PE-ready immediately; issued before
                        # the stats-gated matmuls so PE never queues behind) ----
                        xfT = [cst.tile([P, M], F32R, tag=f"xfT{cc}", name=f"xfT{cc}")
                               for cc in range(2)]
                        for cc in range(2):
                            for mh in range(2):
                                px = psm.tile([P, 512], dt, tag="psc2", bufs=4)
                                nc.tensor.matmul(px[:], spec1[:, cc * P:(cc + 1) * P],
                                                 evTfar_t[:, mh * 512:(mh + 1) * 512],
                                                 start=True, stop=True)
                                nc.vector.tensor_copy(
                                    out=xfT[cc][:, mh * 512:(mh + 1) * 512], in_=px[:])

                        stat_mq = cst.tile([P, 8], F32R, tag="statmq")
                        nc.vector.tensor_copy(out=stat_mq[:], in_=stat_ps[:])
                        pg = psm.tile([16, 8], dt, tag="mm256", bufs=2)
                        nc.tensor.matmul(pg[:], gsum_t, stat_mq[:], start=True, stop=True)
                        inv = 1.0 / (M * 8)
                        # mu lands directly in stats_sb (zero-filled early so
                        # the writes race nothing); rstd is written by ACT
                        stats_sb = cst.tile([P, 4], F32R, tag="stats")
                        nc.vector.tensor_copy(out=stats_sb[:], in_=zeros8[:, 0:4])
                        nc.vector.tensor_scalar_mul(stats_sb[0:16, 0:1], pg[:, 0:1],
                                                    inv)
                        nc.vector.tensor_scalar_mul(stats_sb[0:16, 1:2], pg[:, 2:3],
                                                    inv)
                        ms = cst.tile([16, 2], dt, tag="ms")
                        nc.vector.tensor_scalar_mul(ms[:, 0:1], pg[:, 4:5], inv)
                        nc.vector.tensor_scalar_mul(ms[:, 1:2], pg[:, 6:7], inv)
                        var = cst.tile([16, 2], dt, tag="var")
                        nc.vector.tensor_tensor(var[:], stats_sb[0:16, 0:2],
                                                stats_sb[0:16, 0:2], MULT)
                        nc.vector.tensor_sub(var[:], ms[:], var[:])
                        nc.vector.tensor_scalar_add(var[:], var[:], EPS)
                        # rstd via DVE-only fast inverse sqrt (bit trick + two
                        # Newton steps): keeps ACT (and its table loads) off
                        # the stats chain entirely
                        U32 = mybir.dt.uint32
                        yr = cst.tile([16, 2], dt, tag="rsqy")
                        nc.vector.tensor_scalar(
                            yr[:].bitcast(U32), var[:].bitcast(U32), 1, None,
                            mybir.AluOpType.logical_shift_right)
                        nc.vector.tensor_tensor(
                            yr[:].bitcast(U32),
                            rsqm_t.bitcast(U32).to_broadcast([16, 2]),
                            yr[:].bitcast(U32), mybir.AluOpType.subtract)
                        # one Newton step off the magic-number seed gives
                        # ~0.2% rstd accuracy -- well inside the error budget
                        # and four dependency hops shorter than two steps
                        tn = cst.tile([16, 2], dt, tag="rsqt")
                        nc.vector.tensor_tensor(tn[:], yr[:], yr[:], MULT)
                        nc.vector.tensor_tensor(tn[:], tn[:], var[:], MULT)
                        nc.vector.tensor_scalar(tn[:], tn[:], -0.5, 1.5,
                                                MULT, ADD)
                        nc.vector.tensor_tensor(stats_sb[0:16, 2:4],
                                                yr[:], tn[:], MULT)
                        pbc = psm.tile([P, 4], dt, tag="mm256", bufs=2)
                        nc.tensor.matmul(pbc[:], gbp_t, stats_sb[:], start=True, stop=True)
                        A = cst.tile([P, 2], dt, tag="gnA")
                        nc.vector.tensor_tensor(A[:], pbc[:, 2:4], gnw_t, MULT)
                        # Bt cols 0:2 = B channel chunks, cols 2:4 stay zero so
                        # a 2-wide moving slice [cin, cin+1] isolates chunk cin
                        Bt = cst.tile([P, 4], F32R, tag="gnB")
                        nc.vector.tensor_copy(out=Bt[:, 2:4], in_=zeros8[:, 0:2])
                        nc.vector.tensor_tensor(Bt[:, 0:2], pbc[:, 0:2], A[:], MULT)
                        nc.vector.tensor_sub(Bt[:, 0:2], gnb_t, Bt[:, 0:2])

                        # ---- fold the GN bias into qkv biases: bX2 = W_X^T B
                        # via transposed matmuls, chunks accumulated in PSUM ----
                        bqk = psm.tile([P, 4], dt, tag="mm256", bufs=2)
                        for cin in range(2):
                            nc.tensor.matmul(bqk[:, 0:2], wq_t[cin],
                                             Bt[:, cin:cin + 2],
                                             start=(cin == 0), stop=(cin == 1))
                        for cin in range(2):
                            nc.tensor.matmul(bqk[:, 2:4], wk_t[cin],
                                             Bt[:, cin:cin + 2],
                                             start=(cin == 0), stop=(cin == 1))
                        bq2 = cst.tile([P, 2], F32R, tag="bq2")
                        nc.vector.tensor_add(bq2[:, 0:1], bqk[:, 0:1], bq_t)
                        bk2 = cst.tile([P, 2], dt, tag="bk2")
                        nc.vector.tensor_add(bk2[:, 0:1], bqk[:, 2:3], bk_t)
                        bkm2 = cst.tile([P, HL], F32R, tag="bkm2")
                        nc.vector.tensor_scalar_mul(bkm2[:], maskq_t, bk2[:, 0:1])
                        # per-head v bias at partition base 0 (stationary and
                        # moving must share base partitions)
                        bvps = psm.tile([D, 2 * HL], dt, tag="mm256", bufs=2)
                        for h in range(HL):
                            for cin in range(2):
                                nc.tensor.matmul(
                                    bvps[:, 2 * h:2 * h + 2],
                                    wv_t[cin][:, h * D:(h + 1) * D],
                                    Bt[:, cin:cin + 2],
                                    start=(cin == 0), stop=(cin == 1))
                        bv2h = cst.tile([D, 2 * HL], F32R, tag="bv2h")
                        nc.vector.tensor_copy(out=bv2h[:], in_=zeros8[0:D, 0:8])
                        for h in range(HL):
                            nc.vector.tensor_add(bv2h[:, 2 * h:2 * h + 1],
                                                 bvps[:, 2 * h:2 * h + 1],
                                                 bvh_t[0:D, h:h + 1])
                        # bo2 = bo/2 + bv2 @ Wo: normalized attention rows sum
                        # to one, so the v-side bias lands as a constant row
                        bo2ps = psm.tile([2, C], dt, tag="mm256", bufs=2)
                        for h in range(HL):
                            nc.tensor.matmul(bo2ps[:], bv2h[0:D, 2 * h:2 * h + 2],
                                             wo_t[h], start=(h == 0), stop=False)
                        nc.tensor.matmul(bo2ps[:], ones2[0:1, 0:2], bo_r[:],
                                         start=False, stop=True)
                        bo2row = cst.tile([1, C], F32R, tag="bo2row")
                        nc.vector.tensor_copy(out=bo2row[:], in_=bo2ps[0:1, :])
                        # broadcast bo2 across partitions once (pre-burst) so
                        # the out-projection tail skips 8 accumulate matmuls
                        bo2ps2 = psm.tile([P, C], dt, tag="mm256", bufs=2)
                        nc.tensor.matmul(bo2ps2[:], ones128[0:1, 0:P], bo2row[:],
                                         start=True, stop=True)
                        bo2f = cst.tile([P, C], F32R, tag="bo2f")
                        nc.vector.tensor_copy(out=bo2f[:], in_=bo2ps2[:])
                        # fold the GN scale into the weights (per-partition
                        # channel scalars)
                        for cin in range(2):
                            nc.vector.tensor_scalar_mul(wq_t[cin], wq_t[cin],
                                                        A[:, cin:cin + 1])
                            nc.vector.tensor_scalar_mul(wk_t[cin], wk_t[cin],
                                                        A[:, cin:cin + 1])
                            nc.vector.tensor_scalar_mul(wv_t[cin], wv_t[cin],
                                                        A[:, cin:cin + 1])

                        # ---- qT / kTpad projections (folded weights) ----
                        qT = cst.tile([P, M], F32R, tag="qT")
                        kTpad = [cst.tile([P, M], F32R, tag=f"kTpad{h}",
                                          name=f"kTpad{h}") for h in range(HL)]
                        for mh in range(2):
                            pq = psm.tile([P, 512], dt, tag="psc2", bufs=4)
                            pk = psm.tile([P, 512], dt, tag="psc2", bufs=4)
                            for cin in range(2):
                                nc.tensor.matmul(pq[:], wq_t[cin],
                                                 xfT[cin][:, mh * 512:(mh + 1) * 512],
                                                 start=(cin == 0), stop=(cin == 1))
                            for cin in range(2):
                                nc.tensor.matmul(pk[:], wk_t[cin],
                                                 xfT[cin][:, mh * 512:(mh + 1) * 512],
                                                 start=(cin == 0), stop=(cin == 1))
                            # qT and head-0 kTpad go through ACT (idle until
                            # the exp burst, which needs exactly these first);
                            # Identity(pk*maskq + bkm2) is the masked write
                            nc.scalar.activation(qT[:, mh * 512:(mh + 1) * 512],
                                                 pq[:], AF.Identity,
                                                 bias=bq2[:, 0:1])
                            nc.scalar.activation(kTpad[0][:, mh * 512:(mh + 1) * 512],
                                                 pk[:], AF.Identity,
                                                 bias=bkm2[:, 0:1],
                                                 scale=maskq_t[:, 0:1])
                            for h in range(1, HL):
                                # masked write: rows of head h get k+bk, others 0
                                nc.vector.scalar_tensor_tensor(
                                    kTpad[h][:, mh * 512:(mh + 1) * 512], pk[:, :],
                                    maskq_t[:, h:h + 1],
                                    bkm2[:, h:h + 1].to_broadcast([P, 512]), MULT, ADD)

                        # ---- v (natural, my-half cols first) into vaug blocks;
                        # bias is folded into bo2row ----
                        for mc in range(NMK):
                            pv = psm.tile([P, C], dt, tag="mm256", bufs=2)
                            for cin in range(2):
                                nc.tensor.matmul(pv[:], xfT[cin][:, mc * P:(mc + 1) * P],
                                                 wv_t[cin],
                                                 start=(cin == 0), stop=(cin == 1))
                            nc.vector.tensor_copy(
                                out=vaug[:, mc, :, 0:D],
                                in_=pv[:, 0:P].rearrange("p (h d) -> p h d", h=HL))

                        # ---- attention: scoresT -> 1024-wide exp -> PV ----
                        # per-head tiles at partition base 0 (psum matmul writes
                        # must start at 0/32/64 and stay in-bounds)
                        # per-half oTh tiles: out-projection chunks 0-3 are
                        # gated only by the q0-half normalization
                        oTh = [[cst.tile([D, 512], F32R, tag=f"oTh{h}_{q}",
                                         name=f"oTh{h}_{q}") for q in range(2)]
                               for h in range(HL)]
                        den_sb = cst.tile([1, HL * M], F32R, tag="densb")
                        atp = [None] * NMK
                        scl = 1.0 / np.sqrt(D)
                        for h in range(HL):
                            # q-major: each query-half runs scores->exp->PV to
                            # completion, so its den/normalize (and, for the
                            # last head, the out-projection of its chunks)
                            # overlaps the other half's exp stream
                            for q in range(2):
                                po = psm.tile([D + 1, 512], dt, tag="po", bufs=2,
                                              name=f"po{h}_{q}")
                                pts = [None] * NMK
                                for mkc in range(NMK):
                                    psc = psm.tile([P, 512], dt, tag="psc2",
                                                   bufs=4,
                                                   name=f"psc{h}_{q}_{mkc}")
                                    nc.tensor.matmul(
                                        psc[:],
                                        kTpad[h][:, mkc * P:(mkc + 1) * P],
                                        qT[:, q * 512:(q + 1) * 512],
                                        start=True, stop=True)
                                    pt = mid.tile([P, 512], F32R, tag="ptile",
                                                  bufs=8, name=f"pt{q}_{mkc}")
                                    # 512-wide exp: one ACT read must stay
                                    # within a single PSUM bank
                                    nc.scalar.activation(pt[:], psc[:],
                                                         AF.Exp, scale=scl)
                                    pts[mkc] = pt
                                    if mkc > 0:
                                        nc.tensor.matmul(
                                            po[:], vaug[:, mkc - 1, h, :],
                                            pts[mkc - 1][:],
                                            start=(mkc - 1 == 0), stop=False)
                                    if h == HL - 1 and mkc % 2 == 0:
                                        # heads 0-2 are normalized: accumulate
                                        # their out-projection (+bo2) per chunk
                                        # in head 3's PE slack
                                        mc = q * 4 + mkc // 2
                                        pp3 = psm.tile([P, C], dt, tag="mm256",
                                                       bufs=2, name=f"pp3_{mc}")
                                        for hh in range(HL - 1):
                                            nc.tensor.matmul(
                                                pp3[:], oTh[hh][mc // 4]
                                                [:, (mc % 4) * P:(mc % 4 + 1) * P],
                                                wo_t[hh], start=(hh == 0),
                                                stop=(hh == HL - 2))
                                        atp[mc] = cst.tile([P, C], F32R,
                                                           tag=f"atp{mc}",
                                                           name=f"atp{mc}")
                                        nc.vector.tensor_tensor(atp[mc][:],
                                                                pp3[:],
                                                                bo2f[:], ADD)
                                nc.tensor.matmul(
                                    po[:], vaug[:, NMK - 1, h, :],
                                    pts[NMK - 1][:], start=False, stop=True)
                                # last head's oTh evac goes to ACT (idle while
                                # the other half keeps exping on DVE's clock)
                                if h == HL - 1:
                                    nc.scalar.copy(oTh[h][q][:], po[0:D, :])
                                else:
                                    nc.vector.tensor_copy(out=oTh[h][q][:],
                                                          in_=po[0:D, :])
                                nc.vector.tensor_copy(
                                    out=den_sb[0:1, h * M + q * 512:
                                               h * M + (q + 1) * 512],
                                    in_=po[D:D + 1, :])
                                # denom broadcast via ones-stationary matmul
                                # (reuses the now-dead po buffers)
                                dbp = psm.tile([D, 512], dt, tag="po", bufs=2,
                                               name=f"dbp{h}_{q}")
                                nc.tensor.matmul(
                                    dbp[:], ones128[0:1, 0:D],
                                    den_sb[0:1, h * M + q * 512:
                                           h * M + (q + 1) * 512],
                                    start=True, stop=True)
                                db = mid.tile([D, 512], dt, tag="denb", bufs=4,
                                              name=f"denb{h}_{q}")
                                nc.vector.reciprocal(db[:], dbp[:])
                                nc.vector.tensor_tensor(
                                    oTh[h][q][:], oTh[h][q][:], db[:], MULT)

                        # ---- out-projection tail: heads 0-2 (+bo2) were
                        # pre-accumulated into atp during head 3's burst; only
                        # head 3's matmul and one add remain per chunk ----
                        zspec_ps = psm.tile([K, C], dt, tag="po", bufs=2)
                        ats = [None] * NMK
                        for mc in range(NMK):
                            pa = psm.tile([P, C], dt, tag="mm256", bufs=2)
                            nc.tensor.matmul(pa[:], oTh[HL - 1][mc // 4]
                                             [:, (mc % 4) * P:(mc % 4 + 1) * P],
                                             wo_t[HL - 1],
                                             start=True, stop=True)
                            at = mid.tile([P, C], F32R, tag="atile", bufs=4,
                                          name=f"at{mc}")
                            nc.vector.tensor_tensor(at[:], pa[:], atp[mc][:], ADD)
                            ats[mc] = at
                            if mc > 0:
                                nc.tensor.matmul(zspec_ps[:], ef_all[:, mc - 1, :],
                                                 ats[mc - 1][:],
                                                 start=(mc - 1 == 0), stop=False)
                        nc.tensor.matmul(zspec_ps[:], ef_all[:, NMK - 1, :],
                                         ats[NMK - 1][:], start=False, stop=True)
                        # fold coef_out*out_w during evacuation; AR carries the
                        # finished bf16 spectrum, directly usable by phase 3
                        spec2loc = cst.tile([K, C], DT3, tag="spec2loc")
                        nc.vector.tensor_tensor(spec2loc[:], coefow[:], zspec_ps[:],
                                                MULT)

                    # AllReduce #2 (pair): pre-scaled bf16 spectrum
                    ar2_in = dram.tile([K, C], DT3, tag="ar2in")
                    ar2_out = dram.tile([K, C], DT3, tag="ar2out")
                    nc.sync.dma_start(ar2_in[:], spec2loc[:])
                    if single or noar:
                        nc.sync.dma_start(ar2_out[:], ar2_in[:])
                    else:
                        nc.gpsimd.collective_compute(
                            "AllReduce", ADD, replica_groups=RG,
                            ins=[ar2_in[:].opt()], outs=[ar2_out[:].opt()])
                    spec2 = cst.tile([K, C], DT3, tag="spec2")
                    nc.sync.dma_start(spec2[:], ar2_out[:])

                if 3 not in phases:
                    nc.gpsimd.dma_start(out_ap[0:P, :], spec1[:].bitcast(F32))
                if 3 in phases:
                    # =============== PHASE 3: from_basis (N-split) ===============
                    with tc.tile_pool(name="p3o", bufs=4) as p3o, \
                         tc.tile_pool(name="ps3", bufs=8, space="PSUM") as ps3:
                        if 2 in phases:
                            # PE warm-up through the AR2 window (see AR1 note)
                            for w in range(56):
                                wt = ps3.tile([P, 2, C], dt, tag="p3",
                                              name=f"warm3_{w}")
                                nc.tensor.matmul(wt[:, 0, :],
                                                 ef_all[:, w % NMK, :],
                                                 evTfar_t[:, 0:C],
                                                 start=True, stop=True)
                        ng = NCH // P3G
                        for g in range(ng):
                            if g >= P3E_BUFS:
                                nc.sync.dma_start(
                                    p3et[g][:],
                                    evT_h[:, g * P3G * P:(g + 1) * P3G * P])
                            et = p3et[g]
                            ot = p3o.tile([P, P3G, C], BF16, tag="out8")
                            for j2 in range(P3G // 2):
                                # two matmuls per one-bank psum tile; paired
                                # evacuation halves the copy instruction count
                                pp = ps3.tile([P, 2, C], dt, tag="p3")
                                for u in range(2):
                                    j = 2 * j2 + u
                                    nc.tensor.matmul(pp[:, u, :],
                                                     et[:, j * P:(j + 1) * P],
                                                     spec2[:], start=True, stop=True)
                                if j2 % 2 == 0:
                                    nc.vector.tensor_copy(
                                        out=ot[:, 2 * j2:2 * j2 + 2, :], in_=pp[:])
                                else:
                                    nc.scalar.copy(ot[:, 2 * j2:2 * j2 + 2, :], pp[:])
                            nc.sync.dma_start(
                                out_ap[g * P3G * P:(g + 1) * P3G * P, :]
                                .rearrange("(p j) c -> p j c", j=P3G),
                                ot[:])

    nc.compile()
    return nc


_PROG = None


def _get_prog():
    global _PROG
    if _PROG is None:
        _PROG = _build()
    return _PROG


def make_in_maps(x, mass, evals, evecs, far_idx, diff_in_t, diff_out_t, gn_w, gn_b,
                 Wq, bq, Wk, bk, Wv, bv, Wo, bo, out_w):
    """Host-side (layout-only) prep of the 8 per-core input dicts."""
    import ml_dtypes
    f32 = np.float32
    np1 = ml_dtypes.bfloat16 if DT1 == BF16 else f32
    np3 = ml_dtypes.bfloat16 if DT3 == BF16 else f32
    asf = lambda a: np.ascontiguousarray(a, dtype=f32)
    x = np.asarray(x, dtype=f32)
    mass = np.asarray(mass, dtype=f32)
    evals = np.asarray(evals, dtype=f32)
    evecs = np.asarray(evecs, dtype=f32)
    far_idx = np.asarray(far_idx)
    gsum_m = np.zeros((P, 16), f32)
    gsum_m[np.arange(P), np.arange(P) // 8] = 1.0
    gbp_m = np.zeros((P, P), f32)
    gbp_m[np.arange(P) // 8, np.arange(P)] = 1.0
    maskq_m = (np.arange(P)[:, None] // D == np.arange(HL)[None, :]).astype(f32)
    konst_m = np.stack([np.ones(512, f32), np.zeros(512, f32)])
    konst_m[1, 0:C] = 0.5 * np.asarray(bo, dtype=f32)
    in_maps = []
    for core in range(8):
        b, half = core // 2, core % 2
        rs = slice(half * NH, (half + 1) * NH)
        hc = slice(half * P, (half + 1) * P)        # my C_ATT columns / heads
        oc = slice((1 - half) * P, (2 - half) * P)  # partner's columns
        fi = far_idx[b]
        ev_far = evecs[b][fi]                       # [M, K]
        pk1_m = np.zeros((P, PK1_W), f32)
        pk1_m[:, 0:NCH] = (mass[b, rs].reshape(NCH // P1G, P, P1G)
                           .transpose(1, 0, 2).reshape(P, NCH))
        pk1_m[:, 128:128 + NMK] = mass[b][fi].reshape(NMK, P).T
        pk1_m[:, 136:136 + HL] = maskq_m
        pk1_m[:, 140] = np.asarray(bk)[hc]
        pk1_m[:, 142] = np.frombuffer(
            np.uint32(0x5F3759DF).tobytes(), dtype=f32)[0]
        pk1_m[0:D, 150:150 + HL] = np.asarray(bv)[hc].reshape(HL, D).T
        pk1_m[:, 144:146] = np.asarray(gn_w).reshape(2, P).T
        pk1_m[:, 146:148] = np.asarray(gn_b).reshape(2, P).T
        pk1_m[:, 148] = evals[b]
        pk1_m[:, 149] = np.asarray(bq)[hc]
        pkr_m = np.concatenate([gsum_m, gbp_m], axis=1)
        pkw_m = np.concatenate(
            [np.asarray(Wq)[:, hc].reshape(2, P, P).transpose(1, 0, 2).reshape(P, 2 * P),
             np.asarray(Wk)[:, hc].reshape(2, P, P).transpose(1, 0, 2).reshape(P, 2 * P),
             np.concatenate([np.asarray(Wv)[:, hc], np.asarray(Wv)[:, oc]], axis=1)
             .reshape(2, P, C).transpose(1, 0, 2).reshape(P, 2 * C)], axis=1)
        pkwo_m = np.asarray(Wo)[hc].reshape(HL, D, C).transpose(1, 0, 2) \
            .reshape(D, HL * C)
        rows_m = np.zeros((6, C), f32)
        rows_m[0] = np.asarray(diff_in_t)
        rows_m[1] = np.asarray(diff_out_t)
        rows_m[2] = np.asarray(out_w)
        rows_m[3, 0:P] = np.asarray(bv)[hc]
        rows_m[4] = 0.5 * np.asarray(bo)
        m = {
            "x_h": np.ascontiguousarray(x[b, rs], dtype=np1),
            "ev_h": np.ascontiguousarray(evecs[b, rs], dtype=np1),
            "evT_h": np.ascontiguousarray(
                evecs[b, rs].T.reshape(K, NCH // P3G, P, P3G)
                .transpose(0, 1, 3, 2).reshape(K, NH), dtype=np3),
            "evfar": asf(ev_far),
            "ident": np.eye(P, dtype=f32),
            "pk1": pk1_m,
            "pkr": pkr_m,
            "pkw": asf(pkw_m),
            "pkwo": asf(pkwo_m),
            "rows": rows_m,
            "konst": konst_m,
        }
        in_maps.append(m)
    return in_maps


def kernel(**inputs):
    nc = _get_prog()
    in_maps = make_in_maps(**inputs)
    res = run_bass_kernel_spmd(nc, in_maps, core_ids=list(range(8)))
    out = np.empty((B, N, C), np.float32)
    for core in range(8):
        b, half = core // 2, core % 2
        out[b, half * NH:(half + 1) * NH] = np.asarray(
            res.results[core]["out"], dtype=np.float32)
    return out

